# revision 32
# baseline (speedup 1.0000x reference)
"""GQA attention kernel for 8 TRN2 NeuronCores.

Sharding: data-parallel over batch (B=2) x tensor-parallel over heads (4-way).
Core i handles batch i//4 and head-shard i%4 (8 query heads = 2 KV groups).
Out-proj is row-sharded; the 4 partial [S,D] outputs per batch are summed on
the host (cheap unshard step), bo added once.

Device kernel (per core, all bf16 matmuls, f32 PSUM):
  QT = Wq_sh.T @ xT          [512, S]   (x pre-transposed on host)
  KT = Wk_sh.T @ kvT, stored twice with complementary zero halves (kt2a/b)
       so QK's stationary operand is always 128 rows (partial-row LDWEIGHTS
       cannot use the PE background weight buffer and costs ~+95ns/matmul)
  V  = kvT.T  @ Wv_sh        [S, 128] -> per-group V_aug [S, 64+1] (ones col)
  per (head, q-chunk 512): score chunks [128 keys, 512 q] computed in PAIRS
  into [128,1024] PSUM tiles -> one exp per off-diagonal pair (no max
  subtraction; scores are O(1)) -> causal mask applied only to the 128-col
  diagonal triangle -> PV accumulate with ones-row giving softmax sums in
  row 64 -> normalize via reciprocal + GpSimd partition_broadcast (PE-free,
  so the next head's QKs never queue behind it) -> out = OT.T @ Wo_sh, bf16
  partials summed on the host.

All DRAM inputs are host-staged into partition-contiguous layouts so every
DMA is a plain 2D copy (16KB/partition runs) - cheap SP descriptor gen and
full HBM bandwidth during the startup phase.
"""

import numpy as np

B, S, D = 2, 2048, 2048
H, G, HD, GS = 32, 8, 64, 4
HPC = 8     # query heads per core
GPC = 2     # kv groups per core
NCORES = 8
SCALE = 0.125  # 1/sqrt(64)

_CACHE = {}


def _build():
    import concourse.bass as bass
    import concourse.tile as tile
    from concourse import bacc, mybir

    f32 = mybir.dt.float32
    bf16 = mybir.dt.bfloat16
    AF = mybir.ActivationFunctionType
    ALU = mybir.AluOpType

    nc = bacc.Bacc("TRN2", target_bir_lowering=False, debug=False,
                   num_devices=NCORES)

    # host-staged, partition-major layouts (see _run)
    xT_d = nc.declare_dram_parameter("xT", [128, 4 * 16 * 512], bf16,
                                     isOutput=False)
    kvT_d = nc.declare_dram_parameter("kvT", [128, 4 * 16 * 512], bf16,
                                      isOutput=False)
    wq_d = nc.declare_dram_parameter("wq", [128, 16 * 512], bf16,
                                     isOutput=False)
    wk_d = nc.declare_dram_parameter("wk", [128, 16 * 128], bf16,
                                     isOutput=False)
    wv_d = nc.declare_dram_parameter("wv", [128, 16 * 128], bf16,
                                     isOutput=False)
    wo_d = nc.declare_dram_parameter("wo", [128, 4 * 2048], bf16,
                                     isOutput=False)
    bq_d = nc.declare_dram_parameter("bq", [128, 4], f32, isOutput=False)
    bk_d = nc.declare_dram_parameter("bk", [128, 1], f32, isOutput=False)
    bvt_d = nc.declare_dram_parameter("bvt", [128, 2 * 64], f32, isOutput=False)
    tri_d = nc.declare_dram_parameter("tri", [128, 128], bf16, isOutput=False)
    out_d = nc.declare_dram_parameter("out", [S, D], bf16, isOutput=True)

    with tile.TileContext(nc) as tc:
        with (
            tc.tile_pool(name="persist", bufs=1) as persist,
            tc.tile_pool(name="stream", bufs=3) as stream,
            tc.tile_pool(name="small", bufs=3) as small,
            tc.tile_pool(name="probs", bufs=3) as probs_pool,
            tc.tile_pool(name="ps_s", bufs=2, space="PSUM") as ps_s,
            tc.tile_pool(name="ps_proj", bufs=2, space="PSUM") as ps_proj,
            tc.tile_pool(name="ps_o", bufs=2, space="PSUM") as ps_o,
        ):
            # ---- resident weight tiles (DMAs emitted in compute order) ----
            wq_sb = persist.tile([128, 16 * 512], bf16, tag="wq")   # chunk c at c*512
            wk_sb = persist.tile([128, 16 * 128], bf16, tag="wk")
            wv_sb = persist.tile([128, 16 * 128], bf16, tag="wv")
            wo_sb = persist.tile([128, 4 * 2048], bf16, tag="wo")
            tri_sb = persist.tile([128, 128], bf16, tag="tri")
            bq_sb = persist.tile([128, 4], f32, tag="bq")
            bk_sb = persist.tile([128, 1], f32, tag="bk")
            bvt_sb = persist.tile([128, 2 * 64], f32, tag="bvt")

            # only what the first KT matmuls need, before kvt0; a small first
            # piece (each dma_start costs the SP sequencer ~1us of serial
            # issue time, so avoid over-splitting)
            nc.sync.dma_start(out=wk_sb[:, 0:512], in_=wk_d[:, 0:512])
            nc.sync.dma_start(out=wk_sb[:, 512:2048], in_=wk_d[:, 512:2048])

            # ---- resident projection outputs ----
            qt_sb = persist.tile([128, 4 * 2048], bf16, tag="qt")   # chunk hc at hc*2048
            # K^T stored twice with complementary zero halves so QK's
            # stationary operand is always a full 128-row tile (partial-row
            # LDWEIGHTS cannot use the background weight buffer and costs
            # ~+95ns per matmul); the zero half annihilates the other head
            # stacked in qt's partitions.
            kt2a_sb = persist.tile([128, S], bf16, tag="kt2a")  # rows 0:64 = K of group 0
            kt2b_sb = persist.tile([128, S], bf16, tag="kt2b")  # rows 64:128 = K of group 1
            vaug_sb = persist.tile([128, 2 * 16 * 65], bf16, tag="vaug")  # [gl*1040+tok*65]
            ot_sb = persist.tile([128, 4 * 2048], bf16, tag="ot")
            # one-time zeroing of the complementary kt2 halves (runs on DVE
            # during the initial DMA wait)
            nc.vector.memset(kt2a_sb[64:128, :], 0.0)
            nc.vector.memset(kt2b_sb[0:64, :], 0.0)
            # all 32 ones-columns of vaug in a single strided memset
            nc.vector.memset(
                vaug_sb.rearrange("p (t j) -> p t j", j=65)[:, :, 64:65], 1.0)

            # ---- chain emitters (as thunk lists for PE-filler interleave) ----
            def kv_chain_thunks(tch):
                """K/V projection for kv token chunk tch: DMA + KT + V."""
                th = []
                state = {}

                def dma():
                    kvt = stream.tile([128, 16 * 512], bf16, tag="xs", name="kvt")
                    if tch == 0:
                        # small first piece so kmm(0) issues asap
                        nc.sync.dma_start(out=kvt[:, 0:512],
                                          in_=kvT_d[:, 0:512])
                        nc.sync.dma_start(out=kvt[:, 512:2048],
                                          in_=kvT_d[:, 512:2048])
                    gs = range(1, 4) if tch == 0 else range(4)
                    for g in gs:
                        nc.sync.dma_start(
                            out=kvt[:, g * 2048:(g + 1) * 2048],
                            in_=kvT_d[:, tch * 8192 + g * 2048:
                                      tch * 8192 + (g + 1) * 2048])
                        if tch == 0 and g == 1:
                            # rest of the kv0-chain deps ride behind piece 0
                            nc.sync.dma_start(out=bk_sb[:], in_=bk_d[:, :])
                            nc.sync.dma_start(out=bvt_sb[:], in_=bvt_d[:, :])
                            nc.sync.dma_start(out=wv_sb[:], in_=wv_d[:, :])
                        if tch == 1 and g == 0:
                            # wo is first needed by out-proj(0), ~attention(1)
                            nc.sync.dma_start(
                                out=wo_sb[:, 0:4096], in_=wo_d[:, 0:4096])
                            nc.sync.dma_start(
                                out=wo_sb[:, 4096:8192], in_=wo_d[:, 4096:8192])
                    state["kvt"] = kvt
                    state["kps"] = ps_proj.tile([128, 512], f32, tag="proj",
                                                name="kps")
                th.append(dma)

                def kmm(c):
                    nc.tensor.matmul(
                        state["kps"][:], lhsT=wk_sb[:, c * 128:(c + 1) * 128],
                        rhs=state["kvt"][:, c * 512:(c + 1) * 512],
                        start=(c == 0), stop=(c == 15))
                    if c == 15:
                        sl = slice(tch * 512, (tch + 1) * 512)
                        nc.vector.tensor_scalar(
                            kt2a_sb[0:64, sl], state["kps"][0:64, :],
                            bk_sb[0:64, 0:1], None, ALU.add)
                        nc.vector.tensor_scalar(
                            kt2b_sb[64:128, sl], state["kps"][64:128, :],
                            bk_sb[64:128, 0:1], None, ALU.add)
                for c in range(16):
                    th.append(lambda c=c: kmm(c))

                def vmm(tt, c):
                    if c == 0:
                        state["vps"] = ps_proj.tile([128, 128], f32, tag="proj",
                                                    name="vps")
                    nc.tensor.matmul(
                        state["vps"][:],
                        lhsT=state["kvt"][:, c * 512 + tt * 128:
                                          c * 512 + (tt + 1) * 128],
                        rhs=wv_sb[:, c * 128:(c + 1) * 128],
                        start=(c == 0), stop=(c == 15))
                    if c == 15:
                        tok = tch * 4 + tt
                        for gl in range(2):
                            base = gl * 1040 + tok * 65
                            nc.vector.tensor_tensor(
                                vaug_sb[:, base:base + 64],
                                state["vps"][:, gl * 64:(gl + 1) * 64],
                                bvt_sb[:, gl * 64:(gl + 1) * 64], ALU.add)
                for tt in range(4):
                    for c in range(0, 16, 4):
                        # 4 small matmuls per thunk (they are ~68ns each)
                        def v4(tt=tt, c0=c):
                            for c in range(c0, c0 + 4):
                                vmm(tt, c)
                        th.append(v4)
                return th

            def q_chain_thunks(qch):
                """Q projection for q chunk qch: DMA + 4 head-chunk chains."""
                th = []
                state = {}

                def dma():
                    xt = stream.tile([128, 16 * 512], bf16, tag="xs", name="xt")
                    if qch == 0:
                        # wq is hc-major: the first head-chunk's 0.5MB slice
                        # lands before xT so attention(h=0, jq=0) does not
                        # wait for the other 1.5MB of wq
                        nc.sync.dma_start(out=wq_sb[:, 0:2048],
                                          in_=wq_d[:, 0:2048])
                    for g in range(4):
                        nc.sync.dma_start(
                            out=xt[:, g * 2048:(g + 1) * 2048],
                            in_=xT_d[:, qch * 8192 + g * 2048:
                                     qch * 8192 + (g + 1) * 2048])
                    if qch == 0:
                        for hcb in range(1, 4):
                            nc.sync.dma_start(
                                out=wq_sb[:, hcb * 2048:(hcb + 1) * 2048],
                                in_=wq_d[:, hcb * 2048:(hcb + 1) * 2048])
                    state["xt"] = xt
                th.append(dma)

                def qmm(hc, c):
                    if c == 0:
                        state["qps"] = ps_proj.tile([128, 512], f32, tag="proj",
                                                    name="qps")
                    nc.tensor.matmul(
                        state["qps"][:],
                        lhsT=wq_sb[:, hc * 2048 + c * 128:
                                   hc * 2048 + (c + 1) * 128],
                        rhs=state["xt"][:, c * 512:(c + 1) * 512],
                        start=(c == 0), stop=(c == 15))
                    if c == 15:
                        nc.vector.tensor_scalar(
                            qt_sb[:, hc * 2048 + qch * 512:
                                  hc * 2048 + (qch + 1) * 512],
                            state["qps"][:], bq_sb[:, hc:hc + 1], None, ALU.add)
                for hc in range(4):
                    for c in range(16):
                        th.append(lambda hc=hc, c=c: qmm(hc, c))
                return th

            def outproj_thunks(jqb):
                """Out-projection for q block jqb (4 q-tiles x 4 col-chunks)."""
                th = []
                state = {}

                def omm(qt_i, cc, c):
                    if c == 0:
                        state["outp"] = ps_proj.tile([128, 512], f32, tag="proj",
                                                     name="outp")
                    nc.tensor.matmul(
                        state["outp"][:],
                        lhsT=ot_sb[:, c * 2048 + qt_i * 128:
                                   c * 2048 + (qt_i + 1) * 128],
                        rhs=wo_sb[:, c * 2048 + cc * 512:c * 2048 + (cc + 1) * 512],
                        start=(c == 0), stop=(c == 3))
                    if c == 3:
                        if cc == 0:
                            state["osb"] = stream.tile([128, 2048], bf16,
                                                       tag="osb", name="osb")
                        dst = state["osb"][:, cc * 512:(cc + 1) * 512]
                        if jqb == 3 and cc % 2 == 0:
                            # final block runs after the last exp: ACT is
                            # idle, so splitting drains across ACT+DVE frees
                            # ps_proj slots at 2x the rate (group starts were
                            # stalling up to 2us on the DVE copy backlog)
                            nc.scalar.activation(dst, state["outp"][:],
                                                 AF.Copy)
                        else:
                            nc.vector.tensor_copy(dst, state["outp"][:])
                        if jqb == 3:
                            # last block: DMA per column chunk so the final
                            # transfer after the last copy is only 128KB
                            nc.sync.dma_start(
                                out=out_d[qt_i * 128:(qt_i + 1) * 128,
                                          cc * 512:(cc + 1) * 512],
                                in_=state["osb"][:, cc * 512:(cc + 1) * 512])
                        elif cc == 3:
                            nc.sync.dma_start(
                                out=out_d[qt_i * 128:(qt_i + 1) * 128, :],
                                in_=state["osb"][:])
                for qt_i in range(jqb * 4, jqb * 4 + 4):
                    for cc in range(4):
                        for c in range(4):
                            th.append(lambda q=qt_i, cc=cc, c=c: omm(q, cc, c))
                return th

            # ---- filler queue machinery ----
            fillers = []
            fpos = [0]

            def pop_filler(n=1):
                while n > 0 and fpos[0] < len(fillers):
                    fillers[fpos[0]]()
                    fpos[0] += 1
                    n -= 1

            def drain_fillers_through(idx):
                while fpos[0] <= idx:
                    fillers[fpos[0]]()
                    fpos[0] += 1

            # ---- attention for one (head, q-chunk) ----
            # key chunks processed in PAIRS sharing a [128,1024] PSUM tile
            # (2 banks): one exp instruction covers a non-diagonal pair, and
            # the 2-pair lookahead gives the QK matmuls a full ~1.3us of
            # slack on the sps slot release, hiding ACT latency + sem delay.
            def attention(h, jq):
                gl = h // 4
                hc, hr = h % 4, gl * 64
                nkc = 4 * jq + 4
                npairs = nkc // 2
                qbase = hc * 2048 + jq * 512
                ops = ps_o.tile([65, 512], f32, tag="ops", name="ops")
                sps_t = {}
                pt_t = {}

                def m_of(kci):
                    return max(0, kci * 128 - jq * 512)

                kt2 = kt2a_sb if gl == 0 else kt2b_sb

                def emit_qk_pair(p):
                    sps = ps_s.tile([128, 1024], f32, tag="sps", name="sps")
                    for half in range(2):
                        kci = 2 * p + half
                        m = m_of(kci)
                        nc.tensor.matmul(
                            sps[:, half * 512 + m:(half + 1) * 512],
                            lhsT=kt2[:, kci * 128:(kci + 1) * 128],
                            rhs=qt_sb[:, qbase + m:qbase + 512],
                            start=True, stop=True)
                    sps_t[p] = sps

                def emit_exp_pair(p):
                    sps = sps_t.pop(p)
                    pt = probs_pool.tile([128, 1024], bf16, tag="pt", name="pt")
                    if 2 * p + 1 < 4 * jq:
                        # fully off-diagonal pair: single wide exp, no mask
                        nc.scalar.activation(pt[:], sps[:], AF.Exp, scale=SCALE)
                    else:
                        for half in range(2):
                            kci = 2 * p + half
                            m = m_of(kci)
                            lo, hi = half * 512 + m, (half + 1) * 512
                            nc.scalar.activation(pt[:, lo:hi], sps[:, lo:hi],
                                                 AF.Exp, scale=SCALE)
                            # only the 128-col diagonal triangle needs masking
                            nc.vector.tensor_tensor(
                                pt[:, lo:lo + 128], pt[:, lo:lo + 128],
                                tri_sb[:, 0:128], ALU.mult)
                    pt_t[p] = pt

                def emit_pv(p, half):
                    pt = pt_t[p]
                    kci = 2 * p + half
                    m = m_of(kci)
                    vbase = gl * 1040 + kci * 65
                    nc.tensor.matmul(
                        ops[:, m:512], lhsT=vaug_sb[:, vbase:vbase + 65],
                        rhs=pt[:, half * 512 + m:(half + 1) * 512],
                        start=(kci == 0), stop=(kci == nkc - 1))
                    if half == 1:
                        pt_t.pop(p)

                emit_qk_pair(0)
                if npairs > 1:
                    emit_qk_pair(1)
                for p in range(npairs):
                    emit_exp_pair(p)
                    pop_filler(1)
                    emit_pv(p, 0)
                    emit_pv(p, 1)
                    # QK of pair p+2 reuses the sps slot exp(p) just read, so
                    # emit it last to give the exp a full iteration of slack
                    if p + 2 < npairs:
                        emit_qk_pair(p + 2)
                    pop_filler(1)
                # normalize: 1/sums broadcast down partitions on GpSimd (no PE
                # involvement, so the next head's QKs never stall behind it)
                rss = small.tile([1, 512], f32, tag="rss", name="rss")
                nc.vector.tensor_copy(rss[:], ops[64:65, :])
                rs = small.tile([1, 512], f32, tag="rs")
                nc.vector.reciprocal_approx_fast(rs[:], rss[:])
                bsf = small.tile([64, 512], f32, tag="bsf", name="bsf")
                nc.gpsimd.partition_broadcast(bsf[:], rs[:], channels=64)
                nc.vector.tensor_tensor(
                    ot_sb[hr:hr + 64, qbase:qbase + 512],
                    ops[0:64, :], bsf[:], ALU.mult)

            # ---- emission schedule ----
            # prologue: KV(0) + Q(0) emitted directly; remaining weight DMAs
            # stream in behind compute
            for t in kv_chain_thunks(0):
                t()
            nc.sync.dma_start(out=bq_sb[:], in_=bq_d[:, :])
            nc.sync.dma_start(out=tri_sb[:], in_=tri_d[:, :])
            q0 = q_chain_thunks(0)
            q0[0]()  # xT/wq DMAs
            # fillers, dependency-safe order; interleaved kv/q so late chains
            # are not front-loaded (flatter PE duty for the power governor)
            group_end = {}
            for name, th in [("kv1", kv_chain_thunks(1)),
                             ("q1", q_chain_thunks(1)),
                             ("kv2", kv_chain_thunks(2)),
                             ("q2", q_chain_thunks(2)),
                             ("kv3", kv_chain_thunks(3)),
                             ("q3", q_chain_thunks(3))]:
                fillers.extend(th)
                group_end[name] = len(fillers) - 1

            # jq=0 attention interleaved with the per-hc Q-projection chains:
            # heads hc and hc+4 need only qt chunk hc, so they can run as
            # soon as that chain lands instead of waiting for all of wq
            for hc in range(4):
                for t in q0[1 + hc * 16:17 + hc * 16]:
                    t()
                attention(hc, 0)
                pop_filler(2)
                attention(hc + 4, 0)
                pop_filler(2)
            fillers.extend(outproj_thunks(0))
            group_end["op0"] = len(fillers) - 1

            for jq in range(1, 4):
                # producers attention(jq) needs must be emitted already
                drain_fillers_through(group_end[f"kv{jq}"])
                drain_fillers_through(group_end[f"q{jq}"])
                for h in range(HPC):
                    attention(h, jq)
                    pop_filler(2)
                # out-proj of this block becomes legal filler now
                fillers.extend(outproj_thunks(jq))
                group_end[f"op{jq}"] = len(fillers) - 1
            pop_filler(len(fillers))
    nc.finalize()
    return nc


def _get_nc():
    if "nc" not in _CACHE:
        _CACHE["nc"] = _build()
    return _CACHE["nc"]


def kernel(**inputs):
    out, _ = _run(inputs, trace=False)
    return out


def _run(inputs, trace=False):
    import ml_dtypes
    from concourse.bass_utils import run_bass_kernel_spmd

    x = np.asarray(inputs["x"], np.float32)
    kv = np.asarray(inputs["kv"], np.float32)
    Wq = np.asarray(inputs["Wq"], np.float32)
    bq = np.asarray(inputs["bq"], np.float32)
    Wk = np.asarray(inputs["Wk"], np.float32)
    bk = np.asarray(inputs["bk"], np.float32)
    Wv = np.asarray(inputs["Wv"], np.float32)
    bv = np.asarray(inputs["bv"], np.float32)
    Wo = np.asarray(inputs["Wo"], np.float32)
    bo = np.asarray(inputs["bo"], np.float32)

    bf = ml_dtypes.bfloat16
    TRI = (np.arange(128)[None, :] >= np.arange(128)[:, None]).astype(bf)

    # head-dim permutation: chunk c = [local head c | local head 4+c]
    # so each head's Q rows sit at the partition half of its KV group.
    hperm = np.concatenate(
        [np.concatenate([np.arange(c * 64, c * 64 + 64),
                         np.arange((4 + c) * 64, (4 + c) * 64 + 64)])
         for c in range(4)])  # [512] permutation of local head dims

    def stage_act(a):
        # [D, S] -> [128, tch, c, 512]: partition-contiguous DMA layout
        return np.ascontiguousarray(
            a.reshape(16, 128, 4, 512).transpose(1, 2, 0, 3)
        ).reshape(128, 4 * 16 * 512)

    def stage_w(w, cols):
        # [D, cols] -> [128, c, cols]
        return np.ascontiguousarray(
            w.reshape(16, 128, cols).transpose(1, 0, 2)
        ).reshape(128, 16 * cols)

    in_maps = []
    for core in range(NCORES):
        b, t = core // 4, core % 4
        bv_sh = bv[t * 128:(t + 1) * 128]
        bvt = np.broadcast_to(bv_sh[None, :], (128, 128)).astype(np.float32)
        wq_sh = Wq[:, t * 512:(t + 1) * 512][:, hperm]
        wo_sh = Wo[t * 512:(t + 1) * 512, :][hperm, :]
        bq_sh = bq[t * 512:(t + 1) * 512][hperm]
        wo_st = np.ascontiguousarray(
            wo_sh.reshape(4, 128, 2048).transpose(1, 0, 2)
        ).reshape(128, 4 * 2048)
        # wq staged hc-major: [p, hc, c, 128]
        wq_st = np.ascontiguousarray(
            wq_sh.reshape(16, 128, 4, 128).transpose(1, 2, 0, 3)
        ).reshape(128, 16 * 512)
        in_maps.append({
            "xT": stage_act(x[b].T.astype(bf)),
            "kvT": stage_act(kv[b].T.astype(bf)),
            "wq": wq_st.astype(bf),
            "wk": stage_w(Wk[:, t * 128:(t + 1) * 128].astype(bf), 128),
            "wv": stage_w(Wv[:, t * 128:(t + 1) * 128].astype(bf), 128),
            "wo": wo_st.astype(bf),
            "bq": np.ascontiguousarray(bq_sh.reshape(4, 128).T),
            "bk": bk[t * 128:(t + 1) * 128].reshape(128, 1).copy(),
            "bvt": np.ascontiguousarray(bvt),
            "tri": TRI,
        })

    nc = _get_nc()
    res = run_bass_kernel_spmd(nc, in_maps, core_ids=list(range(NCORES)),
                               trace=trace)
    parts = [np.asarray(res.results[i]["out"]).astype(np.float32)
             for i in range(NCORES)]
    out = np.stack([parts[0] + parts[1] + parts[2] + parts[3],
                    parts[4] + parts[5] + parts[6] + parts[7]])
    out += bo[None, None, :]
    return out.astype(np.float32), res


# revision 35
# speedup vs baseline: 1.1983x; 1.1983x over previous
"""GQA attention kernel for 8 TRN2 NeuronCores.

Sharding: data-parallel over batch (B=2) x tensor-parallel over heads (4-way).
Core i handles batch i//4 and head-shard i%4 (8 query heads = 2 KV groups).
Out-proj is row-sharded; the 4 partial [S,D] outputs per batch are summed on
the host (cheap unshard step), bo added once.

Device kernel (per core, all bf16 matmuls, f32 PSUM):
  QT = Wq_sh.T @ xT          [512, S]   (x pre-transposed on host)
  KT = Wk_sh.T @ kvT, stored twice with complementary zero halves (kt2a/b)
       so QK's stationary operand is always 128 rows (partial-row LDWEIGHTS
       cannot use the PE background weight buffer and costs ~+95ns/matmul)
  V  = kvT.T  @ Wv_sh        [S, 128] -> per-group V_aug [S, 64+1] (ones col)
  per (head, q-chunk 512): score chunks [128 keys, 512 q] computed in PAIRS
  into [128,1024] PSUM tiles -> one exp per off-diagonal pair (no max
  subtraction; scores are O(1)) -> causal mask applied only to the 128-col
  diagonal triangle -> PV accumulate with ones-row giving softmax sums in
  row 64 -> normalize via reciprocal + GpSimd partition_broadcast (PE-free,
  so the next head's QKs never queue behind it) -> out = OT.T @ Wo_sh, bf16
  partials summed on the host.

All DRAM inputs are host-staged into partition-contiguous layouts so every
DMA is a plain 2D copy (16KB/partition runs) - cheap SP descriptor gen and
full HBM bandwidth during the startup phase.
"""

import numpy as np

B, S, D = 2, 2048, 2048
H, G, HD, GS = 32, 8, 64, 4
HPC = 8     # query heads per core
GPC = 2     # kv groups per core
NCORES = 8
SCALE = 0.125  # 1/sqrt(64)

_CACHE = {}


def _build():
    import concourse.bass as bass
    import concourse.tile as tile
    from concourse import bacc, mybir

    f32 = mybir.dt.float32
    bf16 = mybir.dt.bfloat16
    AF = mybir.ActivationFunctionType
    ALU = mybir.AluOpType

    nc = bacc.Bacc("TRN2", target_bir_lowering=False, debug=False,
                   num_devices=NCORES)

    # host-staged, partition-major layouts (see _run)
    xT_d = nc.declare_dram_parameter("xT", [128, 4 * 16 * 512], bf16,
                                     isOutput=False)
    kvT_d = nc.declare_dram_parameter("kvT", [128, 4 * 16 * 512], bf16,
                                      isOutput=False)
    wq_d = nc.declare_dram_parameter("wq", [128, 16 * 512], bf16,
                                     isOutput=False)
    wk_d = nc.declare_dram_parameter("wk", [128, 16 * 128], bf16,
                                     isOutput=False)
    wv_d = nc.declare_dram_parameter("wv", [128, 16 * 128], bf16,
                                     isOutput=False)
    wo_d = nc.declare_dram_parameter("wo", [128, 4 * 2048], bf16,
                                     isOutput=False)
    bq_d = nc.declare_dram_parameter("bq", [128, 4], f32, isOutput=False)
    bk_d = nc.declare_dram_parameter("bk", [128, 1], f32, isOutput=False)
    bvt_d = nc.declare_dram_parameter("bvt", [128, 2 * 64], f32, isOutput=False)
    tri_d = nc.declare_dram_parameter("tri", [128, 128], bf16, isOutput=False)
    out_d = nc.declare_dram_parameter("out", [S, D], bf16, isOutput=True)

    with tile.TileContext(nc) as tc:
        with (
            tc.tile_pool(name="persist", bufs=1) as persist,
            tc.tile_pool(name="stream", bufs=3) as stream,
            tc.tile_pool(name="small", bufs=3) as small,
            tc.tile_pool(name="probs", bufs=3) as probs_pool,
            tc.tile_pool(name="ps_s", bufs=2, space="PSUM") as ps_s,
            tc.tile_pool(name="ps_proj", bufs=2, space="PSUM") as ps_proj,
            tc.tile_pool(name="ps_o", bufs=2, space="PSUM") as ps_o,
        ):
            # ---- resident weight tiles (DMAs emitted in compute order) ----
            wq_sb = persist.tile([128, 16 * 512], bf16, tag="wq")   # chunk c at c*512
            wk_sb = persist.tile([128, 16 * 128], bf16, tag="wk")
            wv_sb = persist.tile([128, 16 * 128], bf16, tag="wv")
            wo_sb = persist.tile([128, 4 * 2048], bf16, tag="wo")
            tri_sb = persist.tile([128, 128], bf16, tag="tri")
            bq_sb = persist.tile([128, 4], f32, tag="bq")
            bk_sb = persist.tile([128, 1], f32, tag="bk")
            bvt_sb = persist.tile([128, 2 * 64], f32, tag="bvt")

            # each dma_start fans out over all 16 DMA engines and the queue
            # drains ~FIFO, so only kmm(0)'s two small pieces go first; the
            # wk remainder is emitted after kvt's first pieces (see kv dma)
            nc.sync.dma_start(out=wk_sb[:, 0:512], in_=wk_d[:, 0:512])

            # ---- resident projection outputs ----
            qt_sb = persist.tile([128, 4 * 2048], bf16, tag="qt")   # chunk hc at hc*2048
            # K^T stored twice with complementary zero halves so QK's
            # stationary operand is always a full 128-row tile (partial-row
            # LDWEIGHTS cannot use the background weight buffer and costs
            # ~+95ns per matmul); the zero half annihilates the other head
            # stacked in qt's partitions.
            kt2a_sb = persist.tile([128, S], bf16, tag="kt2a")  # rows 0:64 = K of group 0
            kt2b_sb = persist.tile([128, S], bf16, tag="kt2b")  # rows 64:128 = K of group 1
            vaug_sb = persist.tile([128, 2 * 16 * 65], bf16, tag="vaug")  # [gl*1040+tok*65]
            ot_sb = persist.tile([128, 4 * 2048], bf16, tag="ot")
            # one-time zeroing of the complementary kt2 halves (runs on DVE
            # during the initial DMA wait)
            nc.vector.memset(kt2a_sb[64:128, :], 0.0)
            nc.vector.memset(kt2b_sb[0:64, :], 0.0)
            # all 32 ones-columns of vaug in a single strided memset
            nc.vector.memset(
                vaug_sb.rearrange("p (t j) -> p t j", j=65)[:, :, 64:65], 1.0)

            # ---- chain emitters (as thunk lists for PE-filler interleave) ----
            def kv_chain_thunks(tch):
                """K/V projection for kv token chunk tch: DMA + KT + V."""
                th = []
                state = {}

                def dma():
                    kvt = stream.tile([128, 16 * 512], bf16, tag="xs", name="kvt")
                    if tch == 0:
                        # small first piece so kmm(0) issues asap
                        nc.sync.dma_start(out=kvt[:, 0:512],
                                          in_=kvT_d[:, 0:512])
                        nc.sync.dma_start(out=kvt[:, 512:2048],
                                          in_=kvT_d[:, 512:2048])
                        nc.sync.dma_start(out=wk_sb[:, 512:2048],
                                          in_=wk_d[:, 512:2048])
                    gs = range(1, 4) if tch == 0 else range(4)
                    for g in gs:
                        nc.sync.dma_start(
                            out=kvt[:, g * 2048:(g + 1) * 2048],
                            in_=kvT_d[:, tch * 8192 + g * 2048:
                                      tch * 8192 + (g + 1) * 2048])
                        if tch == 0 and g == 1:
                            # rest of the kv0-chain deps ride behind piece 0
                            nc.sync.dma_start(out=bk_sb[:], in_=bk_d[:, :])
                            nc.sync.dma_start(out=bvt_sb[:], in_=bvt_d[:, :])
                            nc.sync.dma_start(out=wv_sb[:], in_=wv_d[:, :])
                        if tch == 1 and g == 0:
                            # wo is first needed by out-proj(0), ~attention(1)
                            nc.sync.dma_start(
                                out=wo_sb[:, 0:4096], in_=wo_d[:, 0:4096])
                            nc.sync.dma_start(
                                out=wo_sb[:, 4096:8192], in_=wo_d[:, 4096:8192])
                    state["kvt"] = kvt
                    state["kps"] = ps_proj.tile([128, 512], f32, tag="proj",
                                                name="kps")
                th.append(dma)

                def kmm(c):
                    nc.tensor.matmul(
                        state["kps"][:], lhsT=wk_sb[:, c * 128:(c + 1) * 128],
                        rhs=state["kvt"][:, c * 512:(c + 1) * 512],
                        start=(c == 0), stop=(c == 15))
                    if c == 15:
                        sl = slice(tch * 512, (tch + 1) * 512)
                        nc.vector.tensor_scalar(
                            kt2a_sb[0:64, sl], state["kps"][0:64, :],
                            bk_sb[0:64, 0:1], None, ALU.add)
                        nc.vector.tensor_scalar(
                            kt2b_sb[64:128, sl], state["kps"][64:128, :],
                            bk_sb[64:128, 0:1], None, ALU.add)
                for c in range(16):
                    th.append(lambda c=c: kmm(c))

                def vmm(tt, c):
                    if c == 0:
                        state["vps"] = ps_proj.tile([128, 128], f32, tag="proj",
                                                    name="vps")
                    nc.tensor.matmul(
                        state["vps"][:],
                        lhsT=state["kvt"][:, c * 512 + tt * 128:
                                          c * 512 + (tt + 1) * 128],
                        rhs=wv_sb[:, c * 128:(c + 1) * 128],
                        start=(c == 0), stop=(c == 15))
                    if c == 15:
                        tok = tch * 4 + tt
                        for gl in range(2):
                            base = gl * 1040 + tok * 65
                            nc.vector.tensor_tensor(
                                vaug_sb[:, base:base + 64],
                                state["vps"][:, gl * 64:(gl + 1) * 64],
                                bvt_sb[:, gl * 64:(gl + 1) * 64], ALU.add)
                for tt in range(4):
                    for c in range(0, 16, 4):
                        # 4 small matmuls per thunk (they are ~68ns each)
                        def v4(tt=tt, c0=c):
                            for c in range(c0, c0 + 4):
                                vmm(tt, c)
                        th.append(v4)
                return th

            def q_chain_thunks(qch):
                """Q projection for q chunk qch: DMA + 4 head-chunk chains."""
                th = []
                state = {}

                def dma():
                    xt = stream.tile([128, 16 * 512], bf16, tag="xs", name="xt")
                    if qch == 0:
                        # wq is hc-major: the first head-chunk's 0.5MB slice
                        # lands before xT so attention(h=0, jq=0) does not
                        # wait for the other 1.5MB of wq
                        nc.sync.dma_start(out=wq_sb[:, 0:2048],
                                          in_=wq_d[:, 0:2048])
                    for g in range(4):
                        nc.sync.dma_start(
                            out=xt[:, g * 2048:(g + 1) * 2048],
                            in_=xT_d[:, qch * 8192 + g * 2048:
                                     qch * 8192 + (g + 1) * 2048])
                    if qch == 0:
                        for hcb in range(1, 4):
                            nc.sync.dma_start(
                                out=wq_sb[:, hcb * 2048:(hcb + 1) * 2048],
                                in_=wq_d[:, hcb * 2048:(hcb + 1) * 2048])
                    state["xt"] = xt
                th.append(dma)

                def qmm(hc, c):
                    if c == 0:
                        state["qps"] = ps_proj.tile([128, 512], f32, tag="proj",
                                                    name="qps")
                    nc.tensor.matmul(
                        state["qps"][:],
                        lhsT=wq_sb[:, hc * 2048 + c * 128:
                                   hc * 2048 + (c + 1) * 128],
                        rhs=state["xt"][:, c * 512:(c + 1) * 512],
                        start=(c == 0), stop=(c == 15))
                    if c == 15:
                        nc.vector.tensor_scalar(
                            qt_sb[:, hc * 2048 + qch * 512:
                                  hc * 2048 + (qch + 1) * 512],
                            state["qps"][:], bq_sb[:, hc:hc + 1], None, ALU.add)
                for hc in range(4):
                    for c in range(16):
                        th.append(lambda hc=hc, c=c: qmm(hc, c))
                return th

            def outproj_thunks(jqb):
                """Out-projection for q block jqb (4 q-tiles x 4 col-chunks)."""
                th = []
                state = {}

                def omm(qt_i, cc, c):
                    if c == 0:
                        state["outp"] = ps_proj.tile([128, 512], f32, tag="proj",
                                                     name="outp")
                    nc.tensor.matmul(
                        state["outp"][:],
                        lhsT=ot_sb[:, c * 2048 + qt_i * 128:
                                   c * 2048 + (qt_i + 1) * 128],
                        rhs=wo_sb[:, c * 2048 + cc * 512:c * 2048 + (cc + 1) * 512],
                        start=(c == 0), stop=(c == 3))
                    if c == 3:
                        if cc == 0:
                            state["osb"] = stream.tile([128, 2048], bf16,
                                                       tag="osb", name="osb")
                        nc.vector.tensor_copy(
                            state["osb"][:, cc * 512:(cc + 1) * 512],
                            state["outp"][:])
                        if jqb == 3:
                            # last block: DMA per column chunk so the final
                            # transfer after the last copy is only 128KB
                            nc.sync.dma_start(
                                out=out_d[qt_i * 128:(qt_i + 1) * 128,
                                          cc * 512:(cc + 1) * 512],
                                in_=state["osb"][:, cc * 512:(cc + 1) * 512])
                        elif cc == 3:
                            nc.sync.dma_start(
                                out=out_d[qt_i * 128:(qt_i + 1) * 128, :],
                                in_=state["osb"][:])
                for qt_i in range(jqb * 4, jqb * 4 + 4):
                    for cc in range(4):
                        for c in range(4):
                            th.append(lambda q=qt_i, cc=cc, c=c: omm(q, cc, c))
                return th

            # ---- filler queue machinery ----
            fillers = []
            fpos = [0]

            def pop_filler(n=1):
                while n > 0 and fpos[0] < len(fillers):
                    fillers[fpos[0]]()
                    fpos[0] += 1
                    n -= 1

            def drain_fillers_through(idx):
                while fpos[0] <= idx:
                    fillers[fpos[0]]()
                    fpos[0] += 1

            # ---- attention for one (head, q-chunk) ----
            # key chunks processed in PAIRS sharing a [128,1024] PSUM tile
            # (2 banks): one exp instruction covers a non-diagonal pair, and
            # the 2-pair lookahead gives the QK matmuls a full ~1.3us of
            # slack on the sps slot release, hiding ACT latency + sem delay.
            def attention(h, jq):
                gl = h // 4
                hc, hr = h % 4, gl * 64
                nkc = 4 * jq + 4
                npairs = nkc // 2
                qbase = hc * 2048 + jq * 512
                ops = ps_o.tile([65, 512], f32, tag="ops", name="ops")
                sps_t = {}
                pt_t = {}

                def m_of(kci):
                    return max(0, kci * 128 - jq * 512)

                kt2 = kt2a_sb if gl == 0 else kt2b_sb

                def emit_qk_pair(p):
                    sps = ps_s.tile([128, 1024], f32, tag="sps", name="sps")
                    for half in range(2):
                        kci = 2 * p + half
                        m = m_of(kci)
                        nc.tensor.matmul(
                            sps[:, half * 512 + m:(half + 1) * 512],
                            lhsT=kt2[:, kci * 128:(kci + 1) * 128],
                            rhs=qt_sb[:, qbase + m:qbase + 512],
                            start=True, stop=True)
                    sps_t[p] = sps

                def emit_exp_pair(p):
                    sps = sps_t.pop(p)
                    pt = probs_pool.tile([128, 1024], bf16, tag="pt", name="pt")
                    if 2 * p + 1 < 4 * jq:
                        # fully off-diagonal pair: single wide exp, no mask
                        nc.scalar.activation(pt[:], sps[:], AF.Exp, scale=SCALE)
                    else:
                        for half in range(2):
                            kci = 2 * p + half
                            m = m_of(kci)
                            lo, hi = half * 512 + m, (half + 1) * 512
                            nc.scalar.activation(pt[:, lo:hi], sps[:, lo:hi],
                                                 AF.Exp, scale=SCALE)
                            # only the 128-col diagonal triangle needs masking
                            nc.vector.tensor_tensor(
                                pt[:, lo:lo + 128], pt[:, lo:lo + 128],
                                tri_sb[:, 0:128], ALU.mult)
                    pt_t[p] = pt

                def emit_pv(p, half):
                    pt = pt_t[p]
                    kci = 2 * p + half
                    m = m_of(kci)
                    vbase = gl * 1040 + kci * 65
                    nc.tensor.matmul(
                        ops[:, m:512], lhsT=vaug_sb[:, vbase:vbase + 65],
                        rhs=pt[:, half * 512 + m:(half + 1) * 512],
                        start=(kci == 0), stop=(kci == nkc - 1))
                    if half == 1:
                        pt_t.pop(p)

                emit_qk_pair(0)
                if npairs > 1:
                    emit_qk_pair(1)
                for p in range(npairs):
                    emit_exp_pair(p)
                    pop_filler(1)
                    emit_pv(p, 0)
                    emit_pv(p, 1)
                    # QK of pair p+2 reuses the sps slot exp(p) just read, so
                    # emit it last to give the exp a full iteration of slack
                    if p + 2 < npairs:
                        emit_qk_pair(p + 2)
                    pop_filler(1)
                # normalize: 1/sums broadcast down partitions on GpSimd (no PE
                # involvement, so the next head's QKs never stall behind it)
                rss = small.tile([1, 512], f32, tag="rss", name="rss")
                nc.vector.tensor_copy(rss[:], ops[64:65, :])
                rs = small.tile([1, 512], f32, tag="rs")
                nc.vector.reciprocal_approx_fast(rs[:], rss[:])
                bsf = small.tile([64, 512], f32, tag="bsf", name="bsf")
                nc.gpsimd.partition_broadcast(bsf[:], rs[:], channels=64)
                nc.vector.tensor_tensor(
                    ot_sb[hr:hr + 64, qbase:qbase + 512],
                    ops[0:64, :], bsf[:], ALU.mult)

            # ---- emission schedule ----
            # prologue: KV(0) + Q(0) emitted directly; remaining weight DMAs
            # stream in behind compute
            for t in kv_chain_thunks(0):
                t()
            nc.sync.dma_start(out=bq_sb[:], in_=bq_d[:, :])
            nc.sync.dma_start(out=tri_sb[:], in_=tri_d[:, :])
            q0 = q_chain_thunks(0)
            q0[0]()  # xT/wq DMAs
            # fillers, dependency-safe order; interleaved kv/q so late chains
            # are not front-loaded (flatter PE duty for the power governor)
            group_end = {}
            for name, th in [("kv1", kv_chain_thunks(1)),
                             ("q1", q_chain_thunks(1)),
                             ("kv2", kv_chain_thunks(2)),
                             ("q2", q_chain_thunks(2)),
                             ("kv3", kv_chain_thunks(3)),
                             ("q3", q_chain_thunks(3))]:
                fillers.extend(th)
                group_end[name] = len(fillers) - 1

            # jq=0 attention interleaved with the per-hc Q-projection chains:
            # heads hc and hc+4 need only qt chunk hc, so they can run as
            # soon as that chain lands instead of waiting for all of wq
            for hc in range(4):
                for t in q0[1 + hc * 16:17 + hc * 16]:
                    t()
                attention(hc, 0)
                pop_filler(2)
                attention(hc + 4, 0)
                pop_filler(2)
            fillers.extend(outproj_thunks(0))
            group_end["op0"] = len(fillers) - 1

            for jq in range(1, 4):
                # producers attention(jq) needs must be emitted already
                drain_fillers_through(group_end[f"kv{jq}"])
                drain_fillers_through(group_end[f"q{jq}"])
                for h in range(HPC):
                    attention(h, jq)
                    pop_filler(2)
                # out-proj of this block becomes legal filler now
                fillers.extend(outproj_thunks(jq))
                group_end[f"op{jq}"] = len(fillers) - 1
            pop_filler(len(fillers))
    nc.finalize()
    return nc


def _get_nc():
    if "nc" not in _CACHE:
        _CACHE["nc"] = _build()
    return _CACHE["nc"]


def kernel(**inputs):
    out, _ = _run(inputs, trace=False)
    return out


def _run(inputs, trace=False):
    import ml_dtypes
    from concourse.bass_utils import run_bass_kernel_spmd

    x = np.asarray(inputs["x"], np.float32)
    kv = np.asarray(inputs["kv"], np.float32)
    Wq = np.asarray(inputs["Wq"], np.float32)
    bq = np.asarray(inputs["bq"], np.float32)
    Wk = np.asarray(inputs["Wk"], np.float32)
    bk = np.asarray(inputs["bk"], np.float32)
    Wv = np.asarray(inputs["Wv"], np.float32)
    bv = np.asarray(inputs["bv"], np.float32)
    Wo = np.asarray(inputs["Wo"], np.float32)
    bo = np.asarray(inputs["bo"], np.float32)

    bf = ml_dtypes.bfloat16
    TRI = (np.arange(128)[None, :] >= np.arange(128)[:, None]).astype(bf)

    # head-dim permutation: chunk c = [local head c | local head 4+c]
    # so each head's Q rows sit at the partition half of its KV group.
    hperm = np.concatenate(
        [np.concatenate([np.arange(c * 64, c * 64 + 64),
                         np.arange((4 + c) * 64, (4 + c) * 64 + 64)])
         for c in range(4)])  # [512] permutation of local head dims

    def stage_act(a):
        # [D, S] -> [128, tch, c, 512]: partition-contiguous DMA layout
        return np.ascontiguousarray(
            a.reshape(16, 128, 4, 512).transpose(1, 2, 0, 3)
        ).reshape(128, 4 * 16 * 512)

    def stage_w(w, cols):
        # [D, cols] -> [128, c, cols]
        return np.ascontiguousarray(
            w.reshape(16, 128, cols).transpose(1, 0, 2)
        ).reshape(128, 16 * cols)

    in_maps = []
    for core in range(NCORES):
        b, t = core // 4, core % 4
        bv_sh = bv[t * 128:(t + 1) * 128]
        bvt = np.broadcast_to(bv_sh[None, :], (128, 128)).astype(np.float32)
        wq_sh = Wq[:, t * 512:(t + 1) * 512][:, hperm]
        wo_sh = Wo[t * 512:(t + 1) * 512, :][hperm, :]
        bq_sh = bq[t * 512:(t + 1) * 512][hperm]
        wo_st = np.ascontiguousarray(
            wo_sh.reshape(4, 128, 2048).transpose(1, 0, 2)
        ).reshape(128, 4 * 2048)
        # wq staged hc-major: [p, hc, c, 128]
        wq_st = np.ascontiguousarray(
            wq_sh.reshape(16, 128, 4, 128).transpose(1, 2, 0, 3)
        ).reshape(128, 16 * 512)
        in_maps.append({
            "xT": stage_act(x[b].T.astype(bf)),
            "kvT": stage_act(kv[b].T.astype(bf)),
            "wq": wq_st.astype(bf),
            "wk": stage_w(Wk[:, t * 128:(t + 1) * 128].astype(bf), 128),
            "wv": stage_w(Wv[:, t * 128:(t + 1) * 128].astype(bf), 128),
            "wo": wo_st.astype(bf),
            "bq": np.ascontiguousarray(bq_sh.reshape(4, 128).T),
            "bk": bk[t * 128:(t + 1) * 128].reshape(128, 1).copy(),
            "bvt": np.ascontiguousarray(bvt),
            "tri": TRI,
        })

    nc = _get_nc()
    res = run_bass_kernel_spmd(nc, in_maps, core_ids=list(range(NCORES)),
                               trace=trace)
    parts = [np.asarray(res.results[i]["out"]).astype(np.float32)
             for i in range(NCORES)]
    out = np.stack([parts[0] + parts[1] + parts[2] + parts[3],
                    parts[4] + parts[5] + parts[6] + parts[7]])
    out += bo[None, None, :]
    return out.astype(np.float32), res


# revision 36
# speedup vs baseline: 1.2131x; 1.0123x over previous
"""GQA attention kernel for 8 TRN2 NeuronCores.

Sharding: data-parallel over batch (B=2) x tensor-parallel over heads (4-way).
Core i handles batch i//4 and head-shard i%4 (8 query heads = 2 KV groups).
Out-proj is row-sharded; the 4 partial [S,D] outputs per batch are summed on
the host (cheap unshard step), bo added once.

Device kernel (per core, all bf16 matmuls, f32 PSUM):
  QT = Wq_sh.T @ xT          [512, S]   (x pre-transposed on host)
  KT = Wk_sh.T @ kvT, stored twice with complementary zero halves (kt2a/b)
       so QK's stationary operand is always 128 rows (partial-row LDWEIGHTS
       cannot use the PE background weight buffer and costs ~+95ns/matmul)
  V  = kvT.T  @ Wv_sh        [S, 128] -> per-group V_aug [S, 64+1] (ones col)
  per (head, q-chunk 512): score chunks [128 keys, 512 q] computed in PAIRS
  into [128,1024] PSUM tiles -> one exp per off-diagonal pair (no max
  subtraction; scores are O(1)) -> causal mask applied only to the 128-col
  diagonal triangle -> PV accumulate with ones-row giving softmax sums in
  row 64 -> normalize via reciprocal + GpSimd partition_broadcast (PE-free,
  so the next head's QKs never queue behind it) -> out = OT.T @ Wo_sh, bf16
  partials summed on the host.

All DRAM inputs are host-staged into partition-contiguous layouts so every
DMA is a plain 2D copy (16KB/partition runs) - cheap SP descriptor gen and
full HBM bandwidth during the startup phase.
"""

import numpy as np

B, S, D = 2, 2048, 2048
H, G, HD, GS = 32, 8, 64, 4
HPC = 8     # query heads per core
GPC = 2     # kv groups per core
NCORES = 8
SCALE = 0.125  # 1/sqrt(64)

_CACHE = {}


def _build():
    import concourse.bass as bass
    import concourse.tile as tile
    from concourse import bacc, mybir

    f32 = mybir.dt.float32
    bf16 = mybir.dt.bfloat16
    AF = mybir.ActivationFunctionType
    ALU = mybir.AluOpType

    nc = bacc.Bacc("TRN2", target_bir_lowering=False, debug=False,
                   num_devices=NCORES)

    # host-staged, partition-major layouts (see _run)
    xT_d = nc.declare_dram_parameter("xT", [128, 4 * 16 * 512], bf16,
                                     isOutput=False)
    kvT_d = nc.declare_dram_parameter("kvT", [128, 4 * 16 * 512], bf16,
                                      isOutput=False)
    wq_d = nc.declare_dram_parameter("wq", [128, 16 * 512], bf16,
                                     isOutput=False)
    wk_d = nc.declare_dram_parameter("wk", [128, 16 * 128], bf16,
                                     isOutput=False)
    wv_d = nc.declare_dram_parameter("wv", [128, 16 * 128], bf16,
                                     isOutput=False)
    wo_d = nc.declare_dram_parameter("wo", [128, 4 * 2048], bf16,
                                     isOutput=False)
    bq_d = nc.declare_dram_parameter("bq", [128, 4], f32, isOutput=False)
    bk_d = nc.declare_dram_parameter("bk", [128, 1], f32, isOutput=False)
    bvt_d = nc.declare_dram_parameter("bvt", [128, 2 * 64], f32, isOutput=False)
    tri_d = nc.declare_dram_parameter("tri", [128, 128], bf16, isOutput=False)
    out_d = nc.declare_dram_parameter("out", [S, D], bf16, isOutput=True)

    with tile.TileContext(nc) as tc:
        with (
            tc.tile_pool(name="persist", bufs=1) as persist,
            tc.tile_pool(name="stream", bufs=3) as stream,
            tc.tile_pool(name="small", bufs=3) as small,
            tc.tile_pool(name="probs", bufs=3) as probs_pool,
            tc.tile_pool(name="ps_s", bufs=2, space="PSUM") as ps_s,
            tc.tile_pool(name="ps_proj", bufs=2, space="PSUM") as ps_proj,
            tc.tile_pool(name="ps_o", bufs=2, space="PSUM") as ps_o,
        ):
            # ---- resident weight tiles (DMAs emitted in compute order) ----
            wq_sb = persist.tile([128, 16 * 512], bf16, tag="wq")   # chunk c at c*512
            wk_sb = persist.tile([128, 16 * 128], bf16, tag="wk")
            wv_sb = persist.tile([128, 16 * 128], bf16, tag="wv")
            wo_sb = persist.tile([128, 4 * 2048], bf16, tag="wo")
            tri_sb = persist.tile([128, 128], bf16, tag="tri")
            bq_sb = persist.tile([128, 4], f32, tag="bq")
            bk_sb = persist.tile([128, 1], f32, tag="bk")
            bvt_sb = persist.tile([128, 2 * 64], f32, tag="bvt")

            # each dma_start fans out over all 16 DMA engines and the queue
            # drains ~FIFO, so only kmm(0)'s two small pieces go first; the
            # wk remainder is emitted after kvt's first pieces (see kv dma)
            nc.sync.dma_start(out=wk_sb[:, 0:512], in_=wk_d[:, 0:512])

            # ---- resident projection outputs ----
            qt_sb = persist.tile([128, 4 * 2048], bf16, tag="qt")   # chunk hc at hc*2048
            # K^T stored twice with complementary zero halves so QK's
            # stationary operand is always a full 128-row tile (partial-row
            # LDWEIGHTS cannot use the background weight buffer and costs
            # ~+95ns per matmul); the zero half annihilates the other head
            # stacked in qt's partitions.
            kt2a_sb = persist.tile([128, S], bf16, tag="kt2a")  # rows 0:64 = K of group 0
            kt2b_sb = persist.tile([128, S], bf16, tag="kt2b")  # rows 64:128 = K of group 1
            vaug_sb = persist.tile([128, 2 * 16 * 65], bf16, tag="vaug")  # [gl*1040+tok*65]
            ot_sb = persist.tile([128, 4 * 2048], bf16, tag="ot")
            # one-time zeroing of the complementary kt2 halves (runs on DVE
            # during the initial DMA wait)
            nc.vector.memset(kt2a_sb[64:128, :], 0.0)
            nc.vector.memset(kt2b_sb[0:64, :], 0.0)
            # all 32 ones-columns of vaug in a single strided memset
            nc.vector.memset(
                vaug_sb.rearrange("p (t j) -> p t j", j=65)[:, :, 64:65], 1.0)

            # ---- chain emitters (as thunk lists for PE-filler interleave) ----
            def kv_chain_thunks(tch):
                """K/V projection for kv token chunk tch: DMA + KT + V."""
                th = []
                state = {}

                def dma():
                    kvt = stream.tile([128, 16 * 512], bf16, tag="xs", name="kvt")
                    if tch == 0:
                        # small first piece so kmm(0) issues asap
                        nc.sync.dma_start(out=kvt[:, 0:512],
                                          in_=kvT_d[:, 0:512])
                        nc.sync.dma_start(out=kvt[:, 512:2048],
                                          in_=kvT_d[:, 512:2048])
                        nc.sync.dma_start(out=wk_sb[:, 512:2048],
                                          in_=wk_d[:, 512:2048])
                    gs = range(1, 4) if tch == 0 else range(4)
                    for g in gs:
                        nc.sync.dma_start(
                            out=kvt[:, g * 2048:(g + 1) * 2048],
                            in_=kvT_d[:, tch * 8192 + g * 2048:
                                      tch * 8192 + (g + 1) * 2048])
                        if tch == 0 and g == 1:
                            # rest of the kv0-chain deps ride behind piece 0
                            nc.sync.dma_start(out=bk_sb[:], in_=bk_d[:, :])
                            nc.sync.dma_start(out=bvt_sb[:], in_=bvt_d[:, :])
                            nc.sync.dma_start(out=wv_sb[:], in_=wv_d[:, :])
                        if tch == 3 and g == 0:
                            # op0 fillers only pop after all kv/q fillers
                            # (~jq3), so wo can ride this late - keeping the
                            # early bandwidth for xt1/kvt2 which gate the
                            # jq1/jq2 entry drains
                            nc.sync.dma_start(
                                out=wo_sb[:, 0:4096], in_=wo_d[:, 0:4096])
                            nc.sync.dma_start(
                                out=wo_sb[:, 4096:8192], in_=wo_d[:, 4096:8192])
                    state["kvt"] = kvt
                    state["kps"] = ps_proj.tile([128, 512], f32, tag="proj",
                                                name="kps")
                th.append(dma)

                def kmm(c):
                    nc.tensor.matmul(
                        state["kps"][:], lhsT=wk_sb[:, c * 128:(c + 1) * 128],
                        rhs=state["kvt"][:, c * 512:(c + 1) * 512],
                        start=(c == 0), stop=(c == 15))
                    if c == 15:
                        sl = slice(tch * 512, (tch + 1) * 512)
                        nc.vector.tensor_scalar(
                            kt2a_sb[0:64, sl], state["kps"][0:64, :],
                            bk_sb[0:64, 0:1], None, ALU.add)
                        nc.vector.tensor_scalar(
                            kt2b_sb[64:128, sl], state["kps"][64:128, :],
                            bk_sb[64:128, 0:1], None, ALU.add)
                for c in range(16):
                    th.append(lambda c=c: kmm(c))

                def vmm(tt, c):
                    if c == 0:
                        state["vps"] = ps_proj.tile([128, 128], f32, tag="proj",
                                                    name="vps")
                    nc.tensor.matmul(
                        state["vps"][:],
                        lhsT=state["kvt"][:, c * 512 + tt * 128:
                                          c * 512 + (tt + 1) * 128],
                        rhs=wv_sb[:, c * 128:(c + 1) * 128],
                        start=(c == 0), stop=(c == 15))
                    if c == 15:
                        tok = tch * 4 + tt
                        for gl in range(2):
                            base = gl * 1040 + tok * 65
                            nc.vector.tensor_tensor(
                                vaug_sb[:, base:base + 64],
                                state["vps"][:, gl * 64:(gl + 1) * 64],
                                bvt_sb[:, gl * 64:(gl + 1) * 64], ALU.add)
                for tt in range(4):
                    for c in range(0, 16, 4):
                        # 4 small matmuls per thunk (they are ~68ns each)
                        def v4(tt=tt, c0=c):
                            for c in range(c0, c0 + 4):
                                vmm(tt, c)
                        th.append(v4)
                return th

            def q_chain_thunks(qch):
                """Q projection for q chunk qch: DMA + 4 head-chunk chains."""
                th = []
                state = {}

                def dma():
                    xt = stream.tile([128, 16 * 512], bf16, tag="xs", name="xt")
                    if qch == 0:
                        # wq is hc-major: the first head-chunk's 0.5MB slice
                        # lands before xT so attention(h=0, jq=0) does not
                        # wait for the other 1.5MB of wq
                        nc.sync.dma_start(out=wq_sb[:, 0:2048],
                                          in_=wq_d[:, 0:2048])
                    for g in range(4):
                        nc.sync.dma_start(
                            out=xt[:, g * 2048:(g + 1) * 2048],
                            in_=xT_d[:, qch * 8192 + g * 2048:
                                     qch * 8192 + (g + 1) * 2048])
                    if qch == 0:
                        for hcb in range(1, 4):
                            nc.sync.dma_start(
                                out=wq_sb[:, hcb * 2048:(hcb + 1) * 2048],
                                in_=wq_d[:, hcb * 2048:(hcb + 1) * 2048])
                    state["xt"] = xt
                th.append(dma)

                def qmm(hc, c):
                    if c == 0:
                        state["qps"] = ps_proj.tile([128, 512], f32, tag="proj",
                                                    name="qps")
                    nc.tensor.matmul(
                        state["qps"][:],
                        lhsT=wq_sb[:, hc * 2048 + c * 128:
                                   hc * 2048 + (c + 1) * 128],
                        rhs=state["xt"][:, c * 512:(c + 1) * 512],
                        start=(c == 0), stop=(c == 15))
                    if c == 15:
                        nc.vector.tensor_scalar(
                            qt_sb[:, hc * 2048 + qch * 512:
                                  hc * 2048 + (qch + 1) * 512],
                            state["qps"][:], bq_sb[:, hc:hc + 1], None, ALU.add)
                for hc in range(4):
                    for c in range(16):
                        th.append(lambda hc=hc, c=c: qmm(hc, c))
                return th

            def outproj_thunks(jqb):
                """Out-projection for q block jqb (4 q-tiles x 4 col-chunks)."""
                th = []
                state = {}

                def omm(qt_i, cc, c):
                    if c == 0:
                        state["outp"] = ps_proj.tile([128, 512], f32, tag="proj",
                                                     name="outp")
                    nc.tensor.matmul(
                        state["outp"][:],
                        lhsT=ot_sb[:, c * 2048 + qt_i * 128:
                                   c * 2048 + (qt_i + 1) * 128],
                        rhs=wo_sb[:, c * 2048 + cc * 512:c * 2048 + (cc + 1) * 512],
                        start=(c == 0), stop=(c == 3))
                    if c == 3:
                        if cc == 0:
                            state["osb"] = stream.tile([128, 2048], bf16,
                                                       tag="osb", name="osb")
                        nc.vector.tensor_copy(
                            state["osb"][:, cc * 512:(cc + 1) * 512],
                            state["outp"][:])
                        if jqb == 3:
                            # last block: DMA per column chunk so the final
                            # transfer after the last copy is only 128KB
                            nc.sync.dma_start(
                                out=out_d[qt_i * 128:(qt_i + 1) * 128,
                                          cc * 512:(cc + 1) * 512],
                                in_=state["osb"][:, cc * 512:(cc + 1) * 512])
                        elif cc == 3:
                            nc.sync.dma_start(
                                out=out_d[qt_i * 128:(qt_i + 1) * 128, :],
                                in_=state["osb"][:])
                for qt_i in range(jqb * 4, jqb * 4 + 4):
                    for cc in range(4):
                        for c in range(4):
                            th.append(lambda q=qt_i, cc=cc, c=c: omm(q, cc, c))
                return th

            # ---- filler queue machinery ----
            fillers = []
            fpos = [0]

            def pop_filler(n=1):
                while n > 0 and fpos[0] < len(fillers):
                    fillers[fpos[0]]()
                    fpos[0] += 1
                    n -= 1

            def drain_fillers_through(idx):
                while fpos[0] <= idx:
                    fillers[fpos[0]]()
                    fpos[0] += 1

            # ---- attention for one (head, q-chunk) ----
            # key chunks processed in PAIRS sharing a [128,1024] PSUM tile
            # (2 banks): one exp instruction covers a non-diagonal pair, and
            # the 2-pair lookahead gives the QK matmuls a full ~1.3us of
            # slack on the sps slot release, hiding ACT latency + sem delay.
            def attention(h, jq):
                gl = h // 4
                hc, hr = h % 4, gl * 64
                nkc = 4 * jq + 4
                npairs = nkc // 2
                qbase = hc * 2048 + jq * 512
                ops = ps_o.tile([65, 512], f32, tag="ops", name="ops")
                sps_t = {}
                pt_t = {}

                def m_of(kci):
                    return max(0, kci * 128 - jq * 512)

                kt2 = kt2a_sb if gl == 0 else kt2b_sb

                def emit_qk_pair(p):
                    sps = ps_s.tile([128, 1024], f32, tag="sps", name="sps")
                    for half in range(2):
                        kci = 2 * p + half
                        m = m_of(kci)
                        nc.tensor.matmul(
                            sps[:, half * 512 + m:(half + 1) * 512],
                            lhsT=kt2[:, kci * 128:(kci + 1) * 128],
                            rhs=qt_sb[:, qbase + m:qbase + 512],
                            start=True, stop=True)
                    sps_t[p] = sps

                def emit_exp_pair(p):
                    sps = sps_t.pop(p)
                    pt = probs_pool.tile([128, 1024], bf16, tag="pt", name="pt")
                    if 2 * p + 1 < 4 * jq:
                        # fully off-diagonal pair: single wide exp, no mask
                        nc.scalar.activation(pt[:], sps[:], AF.Exp, scale=SCALE)
                    else:
                        for half in range(2):
                            kci = 2 * p + half
                            m = m_of(kci)
                            lo, hi = half * 512 + m, (half + 1) * 512
                            nc.scalar.activation(pt[:, lo:hi], sps[:, lo:hi],
                                                 AF.Exp, scale=SCALE)
                            # only the 128-col diagonal triangle needs masking
                            nc.vector.tensor_tensor(
                                pt[:, lo:lo + 128], pt[:, lo:lo + 128],
                                tri_sb[:, 0:128], ALU.mult)
                    pt_t[p] = pt

                def emit_pv(p, half):
                    pt = pt_t[p]
                    kci = 2 * p + half
                    m = m_of(kci)
                    vbase = gl * 1040 + kci * 65
                    nc.tensor.matmul(
                        ops[:, m:512], lhsT=vaug_sb[:, vbase:vbase + 65],
                        rhs=pt[:, half * 512 + m:(half + 1) * 512],
                        start=(kci == 0), stop=(kci == nkc - 1))
                    if half == 1:
                        pt_t.pop(p)

                emit_qk_pair(0)
                if npairs > 1:
                    emit_qk_pair(1)
                for p in range(npairs):
                    emit_exp_pair(p)
                    pop_filler(1)
                    emit_pv(p, 0)
                    emit_pv(p, 1)
                    # QK of pair p+2 reuses the sps slot exp(p) just read, so
                    # emit it last to give the exp a full iteration of slack
                    if p + 2 < npairs:
                        emit_qk_pair(p + 2)
                    pop_filler(1)
                # normalize: 1/sums broadcast down partitions on GpSimd (no PE
                # involvement, so the next head's QKs never stall behind it)
                rss = small.tile([1, 512], f32, tag="rss", name="rss")
                nc.vector.tensor_copy(rss[:], ops[64:65, :])
                rs = small.tile([1, 512], f32, tag="rs")
                nc.vector.reciprocal_approx_fast(rs[:], rss[:])
                bsf = small.tile([64, 512], f32, tag="bsf", name="bsf")
                nc.gpsimd.partition_broadcast(bsf[:], rs[:], channels=64)
                nc.vector.tensor_tensor(
                    ot_sb[hr:hr + 64, qbase:qbase + 512],
                    ops[0:64, :], bsf[:], ALU.mult)

            # ---- emission schedule ----
            # prologue: KV(0) + Q(0) emitted directly; remaining weight DMAs
            # stream in behind compute
            for t in kv_chain_thunks(0):
                t()
            nc.sync.dma_start(out=bq_sb[:], in_=bq_d[:, :])
            nc.sync.dma_start(out=tri_sb[:], in_=tri_d[:, :])
            q0 = q_chain_thunks(0)
            q0[0]()  # xT/wq DMAs
            # fillers, dependency-safe order; interleaved kv/q so late chains
            # are not front-loaded (flatter PE duty for the power governor)
            group_end = {}
            for name, th in [("kv1", kv_chain_thunks(1)),
                             ("q1", q_chain_thunks(1)),
                             ("kv2", kv_chain_thunks(2)),
                             ("q2", q_chain_thunks(2)),
                             ("kv3", kv_chain_thunks(3)),
                             ("q3", q_chain_thunks(3))]:
                fillers.extend(th)
                group_end[name] = len(fillers) - 1

            # jq=0 attention interleaved with the per-hc Q-projection chains:
            # heads hc and hc+4 need only qt chunk hc, so they can run as
            # soon as that chain lands instead of waiting for all of wq
            for hc in range(4):
                for t in q0[1 + hc * 16:17 + hc * 16]:
                    t()
                attention(hc, 0)
                pop_filler(2)
                attention(hc + 4, 0)
                pop_filler(2)
            fillers.extend(outproj_thunks(0))
            group_end["op0"] = len(fillers) - 1

            for jq in range(1, 4):
                # producers attention(jq) needs must be emitted already
                drain_fillers_through(group_end[f"kv{jq}"])
                drain_fillers_through(group_end[f"q{jq}"])
                for h in range(HPC):
                    attention(h, jq)
                    pop_filler(2)
                # out-proj of this block becomes legal filler now
                fillers.extend(outproj_thunks(jq))
                group_end[f"op{jq}"] = len(fillers) - 1
            pop_filler(len(fillers))
    nc.finalize()
    return nc


def _get_nc():
    if "nc" not in _CACHE:
        _CACHE["nc"] = _build()
    return _CACHE["nc"]


def kernel(**inputs):
    out, _ = _run(inputs, trace=False)
    return out


def _run(inputs, trace=False):
    import ml_dtypes
    from concourse.bass_utils import run_bass_kernel_spmd

    x = np.asarray(inputs["x"], np.float32)
    kv = np.asarray(inputs["kv"], np.float32)
    Wq = np.asarray(inputs["Wq"], np.float32)
    bq = np.asarray(inputs["bq"], np.float32)
    Wk = np.asarray(inputs["Wk"], np.float32)
    bk = np.asarray(inputs["bk"], np.float32)
    Wv = np.asarray(inputs["Wv"], np.float32)
    bv = np.asarray(inputs["bv"], np.float32)
    Wo = np.asarray(inputs["Wo"], np.float32)
    bo = np.asarray(inputs["bo"], np.float32)

    bf = ml_dtypes.bfloat16
    TRI = (np.arange(128)[None, :] >= np.arange(128)[:, None]).astype(bf)

    # head-dim permutation: chunk c = [local head c | local head 4+c]
    # so each head's Q rows sit at the partition half of its KV group.
    hperm = np.concatenate(
        [np.concatenate([np.arange(c * 64, c * 64 + 64),
                         np.arange((4 + c) * 64, (4 + c) * 64 + 64)])
         for c in range(4)])  # [512] permutation of local head dims

    def stage_act(a):
        # [D, S] -> [128, tch, c, 512]: partition-contiguous DMA layout
        return np.ascontiguousarray(
            a.reshape(16, 128, 4, 512).transpose(1, 2, 0, 3)
        ).reshape(128, 4 * 16 * 512)

    def stage_w(w, cols):
        # [D, cols] -> [128, c, cols]
        return np.ascontiguousarray(
            w.reshape(16, 128, cols).transpose(1, 0, 2)
        ).reshape(128, 16 * cols)

    in_maps = []
    for core in range(NCORES):
        b, t = core // 4, core % 4
        bv_sh = bv[t * 128:(t + 1) * 128]
        bvt = np.broadcast_to(bv_sh[None, :], (128, 128)).astype(np.float32)
        wq_sh = Wq[:, t * 512:(t + 1) * 512][:, hperm]
        wo_sh = Wo[t * 512:(t + 1) * 512, :][hperm, :]
        bq_sh = bq[t * 512:(t + 1) * 512][hperm]
        wo_st = np.ascontiguousarray(
            wo_sh.reshape(4, 128, 2048).transpose(1, 0, 2)
        ).reshape(128, 4 * 2048)
        # wq staged hc-major: [p, hc, c, 128]
        wq_st = np.ascontiguousarray(
            wq_sh.reshape(16, 128, 4, 128).transpose(1, 2, 0, 3)
        ).reshape(128, 16 * 512)
        in_maps.append({
            "xT": stage_act(x[b].T.astype(bf)),
            "kvT": stage_act(kv[b].T.astype(bf)),
            "wq": wq_st.astype(bf),
            "wk": stage_w(Wk[:, t * 128:(t + 1) * 128].astype(bf), 128),
            "wv": stage_w(Wv[:, t * 128:(t + 1) * 128].astype(bf), 128),
            "wo": wo_st.astype(bf),
            "bq": np.ascontiguousarray(bq_sh.reshape(4, 128).T),
            "bk": bk[t * 128:(t + 1) * 128].reshape(128, 1).copy(),
            "bvt": np.ascontiguousarray(bvt),
            "tri": TRI,
        })

    nc = _get_nc()
    res = run_bass_kernel_spmd(nc, in_maps, core_ids=list(range(NCORES)),
                               trace=trace)
    parts = [np.asarray(res.results[i]["out"]).astype(np.float32)
             for i in range(NCORES)]
    out = np.stack([parts[0] + parts[1] + parts[2] + parts[3],
                    parts[4] + parts[5] + parts[6] + parts[7]])
    out += bo[None, None, :]
    return out.astype(np.float32), res
